# revision 1
# baseline (speedup 1.0000x reference)
"""CycleConsistencyLoss on 8 Trainium2 NeuronCores (Bass/Tile, SPMD data-parallel).

Math (per batch, clip [M,D], sent [N,D], prefix masks):
  soft_nn(src,tgt): w = softmax_j(-dist(src_i,tgt_j) masked); nn = w @ tgt
  dist = (|s|^2+|t|^2-2 s.t)/D; softmax shift-invariance =>
  w[i,j] prop exp((2 s_i.t_j - |t_j|^2)/D) * mask_j  (row terms cancel)
  index_nn = sum_u u*beta / sum_u beta over tgt2 = src embeddings
  loss_c = mean_b sum_i (index_nn[i]-i)^2 * mask_i / len_b

Device pipeline per (slot, cycle):
  A: dots[t, i] = X^T Y on PE; Et = exp(2/D dots + bias_t) on ACT (1024-wide;
     bias folds -|t|^2/D and -1e4*(1-mask): exp underflows to exact 0)
     nn_num[d,i] += Xn[tb]^T Et ; den[i] += ones^T Et (thin mm, psum row 32)
  C: nns = nn_num * bcast(1/den)  (approx recip + GPSIMD partition_broadcast)
  D: dots2[u,i] = Y[:,ub]^T nns ; Bt = exp(...); [den;num_hi;num_lo] += thin mm
  final: index_nn = num/den; per-(slot,cycle) loss rows -> DRAM; host averages.

Per-slot length specialization: batches sorted by size, slot gets 8 similar
batches across cores; block/chunk counts from the slot max lens (SPMD-safe).
"""
import os
import sys

sys.path.insert(0, "/opt/trn_rl_repo")

import numpy as np
import ml_dtypes

import concourse.bass as bass
import concourse.tile as tile
from concourse import bacc, mybir
from concourse.bass_utils import run_bass_kernel_spmd

F32 = mybir.dt.float32
F32R = mybir.dt.float32r
BF16 = mybir.dt.bfloat16
EXP = mybir.ActivationFunctionType.Exp
ALU = mybir.AluOpType

B, M, N, D = 32, 1024, 1024, 128
NCORES = 8
SLOTS = B // NCORES  # 4
PEN = -1.0e4  # exp(PEN + x) == 0.0 exactly in fp32
USE_BF16 = os.environ.get("CCL_F32R") != "1"  # bf16 matmuls by default

_PROGRAM_CACHE = {}
LAST_RESULT = None


def _plan_chunks(nblk):
    """Split nblk*128 extent into (offset, width) chunks: [512, rest>=256]."""
    ext = nblk * 128
    if ext <= 512:
        return [(0, 512)]
    return [(0, 512), (512, max(256, ext - 512))]


def _emit(nc, tc, ctx, io, plans):
    ts = bass.ts
    scale = 2.0 / D
    DT = BF16 if USE_BF16 else F32R

    const = ctx.enter_context(tc.tile_pool(name="const", bufs=1))
    emb = ctx.enter_context(tc.tile_pool(name="emb", bufs=2))
    etp = ctx.enter_context(tc.tile_pool(name="etp", bufs=6))
    nnp = ctx.enter_context(tc.tile_pool(name="nnp", bufs=4))
    bcp = ctx.enter_context(tc.tile_pool(name="bcp", bufs=2))
    rrp = ctx.enter_context(tc.tile_pool(name="rrp", bufs=2))
    fin = ctx.enter_context(tc.tile_pool(name="fin", bufs=1))

    ps_mm = ctx.enter_context(tc.tile_pool(name="ps_mm", bufs=3, space="PSUM"))
    ps_nn = ctx.enter_context(tc.tile_pool(name="ps_nn", bufs=2, space="PSUM"))
    ps_dn = ctx.enter_context(tc.tile_pool(name="ps_dn", bufs=1, space="PSUM"))
    ps_th = ctx.enter_context(tc.tile_pool(name="ps_th", bufs=2, space="PSUM"))

    thin_w = const.tile([128, M // 128, 3], DT, tag="thin_w")
    nc.sync.dma_start(out=thin_w, in_=io["thin_w"])
    iota_t = const.tile([2 * SLOTS, 2, 512], F32, tag="iota")
    nc.sync.dma_start(out=iota_t, in_=io["iota8"].rearrange("r (q x) -> r q x", q=2))
    masks_t = const.tile([2 * SLOTS, 2, 512], F32, tag="masks")
    nc.sync.dma_start(out=masks_t, in_=io["masks8"].rearrange("r (q x) -> r q x", q=2))
    rlens_t = const.tile([2 * SLOTS, 1], F32, tag="rlens")
    nc.sync.dma_start(out=rlens_t, in_=io["rlens"])

    # staging: [role(den,hi,lo), k, q, x]; memset 1.0 covers never-written cols
    th_sb = fin.tile([3, 2 * SLOTS, 2, 512], F32, tag="th_sb")
    nc.vector.memset(th_sb, 1.0)

    # ---- slot embedding tiles (lazy, emb pool bufs=2 prefetches) ----
    slot_tiles = {}

    def get_slot(s):
        if s in slot_tiles:
            return slot_tiles[s]
        t = {}
        t["ct"] = emb.tile([128, M], DT, tag="ct", name=f"ct{s}")
        nc.sync.dma_start(out=t["ct"], in_=io["cembT"][s])
        t["st"] = emb.tile([128, N], DT, tag="st", name=f"st{s}")
        nc.sync.dma_start(out=t["st"], in_=io["sembT"][s])
        t["cn"] = emb.tile([128, M // 128, D], DT, tag="cn", name=f"cn{s}")
        nc.sync.dma_start(out=t["cn"], in_=io["cembN"][s])
        t["sn"] = emb.tile([128, N // 128, D], DT, tag="sn", name=f"sn{s}")
        nc.sync.dma_start(out=t["sn"], in_=io["sembN"][s])
        t["bias_c"] = emb.tile([128, M // 128], F32, tag="bias_c", name=f"bc{s}")
        nc.sync.dma_start(out=t["bias_c"], in_=io["bias_c"][s])
        t["bias_s"] = emb.tile([128, M // 128], F32, tag="bias_s", name=f"bs{s}")
        nc.sync.dma_start(out=t["bias_s"], in_=io["bias_s"][s])
        slot_tiles[s] = t
        return t

    # ---- build unit list: one unit per (slot, cycle, chunk) ----
    units = []
    for s in range(SLOTS):
        cb, sb = plans[s]
        for c in range(2):
            n_tb = sb if c == 0 else cb
            n_ub = cb if c == 0 else sb
            for q, (off, w) in enumerate(_plan_chunks(n_ub)):
                units.append(dict(s=s, c=c, q=q, off=off, w=w,
                                  n_tb=n_tb, n_ub=n_ub, k=2 * s + c))
    pairs = [units[i:i + 2] for i in range(0, len(units), 2)]

    def a_iter(u, tb, et):
        t = get_slot(u["s"])
        X = t["st"] if u["c"] == 0 else t["ct"]
        Y = t["ct"] if u["c"] == 0 else t["st"]
        Xn = t["sn"] if u["c"] == 0 else t["cn"]
        b_tgt = t["bias_s"] if u["c"] == 0 else t["bias_c"]
        w, off = u["w"], u["off"]
        j = u["j"]
        mm = ps_mm.tile([128, 512], F32, tag="mm", name=f"mmA_{u['k']}_{u['q']}_{tb}")
        nc.tensor.matmul(mm[:, 0:w], lhsT=X[:, ts(tb, 128)],
                         rhs=Y[:, off:off + w], start=True, stop=True)
        nc.scalar.activation(et[:, 0:w], mm[:, 0:w], EXP,
                             bias=b_tgt[:, tb:tb + 1], scale=scale)
        first, last = tb == 0, tb == u["n_tb"] - 1
        nc.tensor.matmul(u["dn"][32 * j:32 * j + 1, 0:w],
                         lhsT=thin_w[:, tb, 0:1], rhs=et[:, 0:w],
                         start=first, stop=last)
        nc.tensor.matmul(u["nn"][:, 0:w], lhsT=Xn[:, tb, :], rhs=et[:, 0:w],
                         start=first, stop=last)

    def c_phase(u):
        w, j = u["w"], u["j"]
        dnc = rrp.tile([1, 512], F32, tag="dnc")
        nc.vector.tensor_copy(dnc[:, 0:w], u["dn"][32 * j:32 * j + 1, 0:w])
        rr = rrp.tile([1, 512], F32, tag="rr")
        nc.vector.reciprocal_approx_fast(out=rr[:, 0:w], in_=dnc[:, 0:w])
        bc = bcp.tile([128, 512], F32, tag="bc")
        nc.gpsimd.partition_broadcast(bc[:, 0:w], rr[:, 0:w])
        nt = nnp.tile([128, 512], DT, tag="nns")
        nc.vector.scalar_tensor_tensor(nt[:, 0:w], in0=u["nn"][:, 0:w],
                                       scalar=1.0, in1=bc[:, 0:w],
                                       op0=ALU.bypass, op1=ALU.mult)
        u["nns"] = nt

    def d_iter(u, ub, bt):
        t = get_slot(u["s"])
        Y = t["ct"] if u["c"] == 0 else t["st"]
        b_src = t["bias_c"] if u["c"] == 0 else t["bias_s"]
        w, j = u["w"], u["j"]
        mm2 = ps_mm.tile([128, 512], F32, tag="mm", name=f"mmD_{u['k']}_{u['q']}_{ub}")
        nc.tensor.matmul(mm2[:, 0:w], lhsT=Y[:, ts(ub, 128)],
                         rhs=u["nns"][:, 0:w], start=True, stop=True)
        nc.scalar.activation(bt[:, 0:w], mm2[:, 0:w], EXP,
                             bias=b_src[:, ub:ub + 1], scale=scale)
        nc.tensor.matmul(u["th"][64 * j:64 * j + 3, 0:w], lhsT=thin_w[:, ub, :],
                         rhs=bt[:, 0:w], start=(ub == 0), stop=(ub == u["n_ub"] - 1))
        if ub == u["n_ub"] - 1:
            nc.vector.tensor_copy(th_sb[:, u["k"], u["q"], 0:w],
                                  u["th"][64 * j:64 * j + 3, 0:w])

    def segment(d_units, a_units, pi):
        if a_units:
            dn = ps_dn.tile([33, 512], F32, tag="dn", name=f"dn_{pi}")
            for j, u in enumerate(a_units):
                u["j"], u["dn"] = j, dn
                u["nn"] = ps_nn.tile([128, 512], F32, tag="nn",
                                     name=f"nn_{u['k']}_{u['q']}")
        if d_units:
            th = ps_th.tile([67, 512], F32, tag="th", name=f"th_{pi}")
            for j, u in enumerate(d_units):
                u["j"], u["th"] = j, th
        n_iter = max([u["n_tb"] for u in a_units] + [u["n_ub"] for u in d_units]
                     + [0])
        for i in range(n_iter):
            for u in a_units:
                if i < u["n_tb"]:
                    et = etp.tile([128, 512], DT, tag="et")
                    a_iter(u, i, et)
            for u in d_units:
                if i < u["n_ub"]:
                    bt = etp.tile([128, 512], DT, tag="et")
                    d_iter(u, i, bt)

    prev = []
    for pi, pair in enumerate(pairs):
        segment(prev, pair, pi)
        for u in pair:
            c_phase(u)
        prev = pair
    segment(prev, [], len(pairs))

    # ---- final ----
    den8 = fin.tile([2 * SLOTS, 2, 512], F32, tag="den8")
    hi8 = fin.tile([2 * SLOTS, 2, 512], F32, tag="hi8")
    lo8 = fin.tile([2 * SLOTS, 2, 512], F32, tag="lo8")
    nc.sync.dma_start(out=den8, in_=th_sb[0:1, :, :, :])
    nc.sync.dma_start(out=hi8, in_=th_sb[1:2, :, :, :])
    nc.sync.dma_start(out=lo8, in_=th_sb[2:3, :, :, :])
    num8 = fin.tile([2 * SLOTS, 2, 512], F32, tag="num8")
    nc.vector.tensor_add(num8, hi8, lo8)
    rden = fin.tile([2 * SLOTS, 2, 512], F32, tag="rden")
    scr = fin.tile([2 * SLOTS, 2, 512], F32, tag="scr")
    nc.vector.reciprocal_approx_accurate(out=rden, in_=den8, scratch=scr)
    idx = fin.tile([2 * SLOTS, 2, 512], F32, tag="idx")
    nc.vector.tensor_mul(idx, num8, rden)
    ierr = fin.tile([2 * SLOTS, 2, 512], F32, tag="ierr")
    nc.vector.tensor_sub(ierr, idx, iota_t)
    tmp = fin.tile([2 * SLOTS, 2, 512], F32, tag="tmp")
    nc.vector.tensor_mul(tmp, ierr, masks_t)
    sq = fin.tile([2 * SLOTS, 2, 512], F32, tag="sq")
    sums = fin.tile([2 * SLOTS, 1], F32, tag="sums")
    nc.vector.scalar_tensor_tensor(sq, in0=tmp, scalar=1.0, in1=ierr,
                                   op0=ALU.bypass, op1=ALU.mult, accum_out=sums)
    loss = fin.tile([2 * SLOTS, 1], F32, tag="loss")
    nc.vector.tensor_mul(loss, sums, rlens_t)
    nc.sync.dma_start(out=io["loss8"], in_=loss)


def _build_program(plans):
    key = (USE_BF16, tuple(plans))
    if key in _PROGRAM_CACHE:
        return _PROGRAM_CACHE[key]
    nc = bacc.Bacc("TRN2", target_bir_lowering=False, debug=False,
                   num_devices=NCORES)
    NB = M // 128
    DT = BF16 if USE_BF16 else F32R
    io = {
        "cembT": nc.dram_tensor("cembT", [SLOTS, D, M], DT, kind="ExternalInput").ap(),
        "sembT": nc.dram_tensor("sembT", [SLOTS, D, N], DT, kind="ExternalInput").ap(),
        "cembN": nc.dram_tensor("cembN", [SLOTS, 128, NB, D], DT, kind="ExternalInput").ap(),
        "sembN": nc.dram_tensor("sembN", [SLOTS, 128, NB, D], DT, kind="ExternalInput").ap(),
        "bias_c": nc.dram_tensor("bias_c", [SLOTS, 128, NB], F32, kind="ExternalInput").ap(),
        "bias_s": nc.dram_tensor("bias_s", [SLOTS, 128, NB], F32, kind="ExternalInput").ap(),
        "thin_w": nc.dram_tensor("thin_w", [128, NB, 3], DT, kind="ExternalInput").ap(),
        "iota8": nc.dram_tensor("iota8", [2 * SLOTS, M], F32, kind="ExternalInput").ap(),
        "masks8": nc.dram_tensor("masks8", [2 * SLOTS, M], F32, kind="ExternalInput").ap(),
        "rlens": nc.dram_tensor("rlens", [2 * SLOTS, 1], F32, kind="ExternalInput").ap(),
        "loss8": nc.dram_tensor("loss8", [2 * SLOTS, 1], F32, kind="ExternalOutput").ap(),
    }
    from contextlib import ExitStack
    with tile.TileContext(nc) as tc:
        with ExitStack() as ctx:
            _emit(nc, tc, ctx, io, plans)
    nc.compile()
    _PROGRAM_CACHE[key] = nc
    return nc


def _host_prep(clip_emb, clip_mask, clip_lens, sent_emb, sent_mask, sent_lens):
    """Sorted batch->$(core,slot) assignment, per-slot plans, per-core inputs."""
    NB = M // 128
    mdt = ml_dtypes.bfloat16 if USE_BF16 else np.float32

    cb_all = np.ceil(clip_lens / 128).astype(int)
    sb_all = np.ceil(sent_lens / 128).astype(int)
    order = np.argsort(-(cb_all + sb_all) * 1000 - cb_all)  # big batches first
    plans = []
    assign = {}  # (core, slot) -> batch
    for s in range(SLOTS):
        grp = order[8 * s:8 * s + 8]
        plans.append((int(cb_all[grp].max()), int(sb_all[grp].max())))
        for core, b in enumerate(grp):
            assign[(core, s)] = int(b)

    sq_c = np.einsum("bmd,bmd->bm", clip_emb, clip_emb)
    sq_s = np.einsum("bnd,bnd->bn", sent_emb, sent_emb)
    bias_c = (-sq_c / D + PEN * (1.0 - clip_mask)).astype(np.float32)
    bias_s = (-sq_s / D + PEN * (1.0 - sent_mask)).astype(np.float32)

    thin_w = np.zeros((128, NB, 3), np.float32)
    thin_w[:, :, 0] = 1.0
    u = np.arange(128)[:, None] + 128 * np.arange(NB)[None, :]
    thin_w[:, :, 1] = (u & ~3).astype(np.float32)   # exact in bf16 (8-bit mantissa)
    thin_w[:, :, 2] = (u & 3).astype(np.float32)
    iota8 = np.broadcast_to(np.arange(M, dtype=np.float32), (2 * SLOTS, M)).copy()

    in_maps = []
    for core in range(NCORES):
        bs = [assign[(core, s)] for s in range(SLOTS)]
        ce = clip_emb[bs]
        se = sent_emb[bs]
        masks8 = np.empty((2 * SLOTS, M), np.float32)
        rlens = np.empty((2 * SLOTS, 1), np.float32)
        for s, b in enumerate(bs):
            masks8[2 * s + 0] = clip_mask[b]
            masks8[2 * s + 1] = sent_mask[b]
            rlens[2 * s + 0] = 1.0 / clip_lens[b]
            rlens[2 * s + 1] = 1.0 / sent_lens[b]
        in_maps.append({
            "cembT": np.ascontiguousarray(ce.transpose(0, 2, 1)).astype(mdt),
            "sembT": np.ascontiguousarray(se.transpose(0, 2, 1)).astype(mdt),
            "cembN": np.ascontiguousarray(
                ce.reshape(SLOTS, NB, 128, D).transpose(0, 2, 1, 3)).astype(mdt),
            "sembN": np.ascontiguousarray(
                se.reshape(SLOTS, NB, 128, D).transpose(0, 2, 1, 3)).astype(mdt),
            "bias_c": np.ascontiguousarray(
                bias_c[bs].reshape(SLOTS, NB, 128).transpose(0, 2, 1)),
            "bias_s": np.ascontiguousarray(
                bias_s[bs].reshape(SLOTS, NB, 128).transpose(0, 2, 1)),
            "thin_w": thin_w.astype(mdt),
            "iota8": iota8,
            "masks8": masks8,
            "rlens": rlens,
        })
    return in_maps, assign, plans


def kernel(clip_emb, clip_mask, clip_lens, sent_emb, sent_mask, sent_lens):
    global LAST_RESULT
    clip_emb = np.asarray(clip_emb, np.float32)
    sent_emb = np.asarray(sent_emb, np.float32)
    clip_mask = np.asarray(clip_mask, np.float32)
    sent_mask = np.asarray(sent_mask, np.float32)
    clip_lens = np.asarray(clip_lens, np.float32)
    sent_lens = np.asarray(sent_lens, np.float32)

    in_maps, _, plans = _host_prep(clip_emb, clip_mask, clip_lens,
                                   sent_emb, sent_mask, sent_lens)
    nc = _build_program(plans)
    res = run_bass_kernel_spmd(nc, in_maps, list(range(NCORES)))
    LAST_RESULT = res

    rows = np.stack([res.results[c]["loss8"].reshape(2 * SLOTS) for c in range(NCORES)])
    clip_loss = rows[:, 0::2].mean()
    sent_loss = rows[:, 1::2].mean()
    return (np.float32(clip_loss), np.float32(sent_loss))



# revision 5
# speedup vs baseline: 1.0170x; 1.0170x over previous
"""CycleConsistencyLoss on 8 Trainium2 NeuronCores (Bass/Tile, SPMD data-parallel).

Math (per batch, clip [M,D], sent [N,D], prefix masks):
  soft_nn(src,tgt): w = softmax_j(-dist(src_i,tgt_j) masked); nn = w @ tgt
  dist = (|s|^2+|t|^2-2 s.t)/D; softmax shift-invariance =>
  w[i,j] prop exp((2 s_i.t_j - |t_j|^2)/D) * mask_j  (row terms cancel)
  index_nn = sum_u u*beta / sum_u beta over tgt2 = src embeddings
  loss_c = mean_b sum_i (index_nn[i]-i)^2 * mask_i / len_b

Device pipeline per (slot, cycle) unit, engines balanced:
  A: dots[t,i] = X^T Y (PE, full-width psum [128,<=1024]); Et = exp (one wide
     ACT op per t-block, bias folds -|t|^2/D - 1e4*(1-mask));
     nn[d,i] += Xn^T Et (PE); dacc += Et (DVE, bf16)
  C: den = partition_all_reduce(dacc) (GPSIMD); rden = recip (DVE);
     nns = nn * rden (DVE, psum->sbuf bf16)
  D: dots2[u,i] = Y^T nns (PE); Bt = exp (ACT); S += Bt, T += u_blk*Bt (DVE)
  fin: den2/num via 2 thin matmuls vs [1|p|128] weights (PE), stage rows
     (GPSIMD copies), batched loss math, DMA out; host averages.

Per-slot length specialization: batches packed into slots minimizing
sum of per-slot (maxcb,maxsb) cost; block counts from slot maxima (SPMD-safe).
"""
import sys

sys.path.insert(0, "/opt/trn_rl_repo")

import numpy as np
import ml_dtypes

import concourse.bass as bass
import concourse.tile as tile
from concourse import bacc, bass_isa, mybir
from concourse.bass_utils import run_bass_kernel_spmd

F32 = mybir.dt.float32
BF16 = mybir.dt.bfloat16
EXP = mybir.ActivationFunctionType.Exp
ALU = mybir.AluOpType

B, M, N, D = 32, 1024, 1024, 128
NB = M // 128
NCORES = 8
SLOTS = B // NCORES  # 4
PEN = -1.0e4  # exp(PEN + x) == 0.0 exactly in fp32
SCALE = 2.0 / D

_PROGRAM_CACHE = {}
LAST_RESULT = None


def _chunks(ie):
    """Split extent ie into <=512-wide chunks (psum-bank sized)."""
    if ie <= 512:
        return [(0, ie)]
    return [(0, 512), (512, ie - 512)]


class U:
    """Per-(slot,cycle) unit state."""

    def __init__(self, s, c, n_tb, n_ub):
        self.s, self.c, self.k = s, c, 2 * s + c
        self.T, self.I = n_tb, n_ub
        self.Ie = 128 * n_ub
        self.pend_a = None
        self.pend_d = None


def _emit(nc, tc, ctx, io, plans):
    ts = bass.ts

    const = ctx.enter_context(tc.tile_pool(name="const", bufs=1))
    emb = ctx.enter_context(tc.tile_pool(name="emb", bufs=2))
    etp = ctx.enter_context(tc.tile_pool(name="etp", bufs=6))
    daccp = ctx.enter_context(tc.tile_pool(name="daccp", bufs=3))
    stp = ctx.enter_context(tc.tile_pool(name="stp", bufs=3))
    nnsp = ctx.enter_context(tc.tile_pool(name="nnsp", bufs=3))
    rrp = ctx.enter_context(tc.tile_pool(name="rrp", bufs=2))
    fin = ctx.enter_context(tc.tile_pool(name="fin", bufs=1))

    ps_mm = ctx.enter_context(tc.tile_pool(name="ps_mm", bufs=2, space="PSUM"))
    ps_nn = ctx.enter_context(tc.tile_pool(name="ps_nn", bufs=2, space="PSUM"))

    thin_w = const.tile([128, 3], BF16, tag="thin_w")
    nc.sync.dma_start(out=thin_w, in_=io["thin_w"])
    iota_t = const.tile([2 * SLOTS, 2, 512], F32, tag="iota")
    nc.sync.dma_start(out=iota_t, in_=io["iota8"].rearrange("r (q x) -> r q x", q=2))
    masks_t = const.tile([2 * SLOTS, 2, 512], F32, tag="masks")
    nc.sync.dma_start(out=masks_t, in_=io["masks8"].rearrange("r (q x) -> r q x", q=2))
    rlens_t = const.tile([2 * SLOTS, 1], F32, tag="rlens")
    nc.sync.dma_start(out=rlens_t, in_=io["rlens"])

    # staging: den2/numA at partitions 0/1, numB at partition 32 (engine ops
    # need start partition in {0,32,64,96}); memset 1.0 covers unwritten cols
    th_sb = fin.tile([33, 2 * SLOTS, 2, 512], F32, tag="th_sb")
    nc.vector.memset(th_sb, 1.0)

    slot_tiles = {}

    def get_slot(s):
        if s in slot_tiles:
            return slot_tiles[s]
        t = {}
        t["ct"] = emb.tile([128, M], BF16, tag="ct", name=f"ct{s}")
        nc.sync.dma_start(out=t["ct"], in_=io["cembT"][s])
        t["st"] = emb.tile([128, N], BF16, tag="st", name=f"st{s}")
        nc.sync.dma_start(out=t["st"], in_=io["sembT"][s])
        t["cn"] = emb.tile([128, NB, D], BF16, tag="cn", name=f"cn{s}")
        nc.sync.dma_start(out=t["cn"], in_=io["cembN"][s])
        t["sn"] = emb.tile([128, NB, D], BF16, tag="sn", name=f"sn{s}")
        nc.sync.dma_start(out=t["sn"], in_=io["sembN"][s])
        t["bias_c"] = emb.tile([128, NB], F32, tag="bias_c", name=f"bc{s}")
        nc.sync.dma_start(out=t["bias_c"], in_=io["bias_c"][s])
        t["bias_s"] = emb.tile([128, NB], F32, tag="bias_s", name=f"bs{s}")
        nc.sync.dma_start(out=t["bias_s"], in_=io["bias_s"][s])
        slot_tiles[s] = t
        return t

    def a_iter(u, tb):
        """dots(tb) + exp; nn/dacc of the PREVIOUS tb (keeps PE off ACT's back)."""
        t = get_slot(u.s)
        X = t["st"] if u.c == 0 else t["ct"]
        Y = t["ct"] if u.c == 0 else t["st"]
        b_tgt = t["bias_s"] if u.c == 0 else t["bias_c"]
        ie = u.Ie
        mm = ps_mm.tile([128, 1024], F32, tag="mm", name=f"mmA_{u.k}_{tb}")
        for off, w in _chunks(ie):
            nc.tensor.matmul(mm[:, off:off + w], lhsT=X[:, ts(tb, 128)],
                             rhs=Y[:, off:off + w], start=True, stop=True)
        et = etp.tile([128, 1024], BF16, tag="et", name=f"etA_{u.k}_{tb}")
        nc.scalar.activation(et[:, 0:ie], mm[:, 0:ie], EXP,
                             bias=b_tgt[:, tb:tb + 1], scale=SCALE)
        a_drain(u)
        u.pend_a = (tb, et)

    def a_drain(u):
        if u.pend_a is None:
            return
        tb, et = u.pend_a
        u.pend_a = None
        t = get_slot(u.s)
        Xn = t["sn"] if u.c == 0 else t["cn"]
        ie = u.Ie
        for off, w in _chunks(ie):
            nc.tensor.matmul(u.nn[:, off:off + w], lhsT=Xn[:, tb, :],
                             rhs=et[:, off:off + w],
                             start=(tb == 0), stop=(tb == u.T - 1))
        if tb == 0:
            nc.vector.tensor_copy(u.dacc[:, 0:ie], et[:, 0:ie])
        else:
            nc.vector.tensor_add(u.dacc[:, 0:ie], u.dacc[:, 0:ie], et[:, 0:ie])

    def c_phase(u):
        ie = u.Ie
        denf = rrp.tile([128, 1024], F32, tag="denf", name=f"denf_{u.k}")
        nc.gpsimd.partition_all_reduce(denf[:, 0:ie], u.dacc[:, 0:ie],
                                       channels=128,
                                       reduce_op=bass_isa.ReduceOp.add)
        rden = rrp.tile([128, 1024], F32, tag="rden", name=f"rden_{u.k}")
        nc.vector.reciprocal_approx_fast(out=rden[:, 0:ie], in_=denf[:, 0:ie])
        nns = nnsp.tile([128, 1024], BF16, tag="nns", name=f"nns_{u.k}")
        nc.vector.scalar_tensor_tensor(nns[:, 0:ie], in0=u.nn[:, 0:ie],
                                       scalar=1.0, in1=rden[:, 0:ie],
                                       op0=ALU.bypass, op1=ALU.mult)
        u.nns = nns

    def d_iter(u, ub):
        t = get_slot(u.s)
        Y = t["ct"] if u.c == 0 else t["st"]
        b_src = t["bias_c"] if u.c == 0 else t["bias_s"]
        ie = u.Ie
        mm2 = ps_mm.tile([128, 1024], F32, tag="mm", name=f"mmD_{u.k}_{ub}")
        for off, w in _chunks(ie):
            nc.tensor.matmul(mm2[:, off:off + w], lhsT=Y[:, ts(ub, 128)],
                             rhs=u.nns[:, off:off + w], start=True, stop=True)
        bt = etp.tile([128, 1024], BF16, tag="et", name=f"btD_{u.k}_{ub}")
        nc.scalar.activation(bt[:, 0:ie], mm2[:, 0:ie], EXP,
                             bias=b_src[:, ub:ub + 1], scale=SCALE)
        d_drain(u)
        u.pend_d = (ub, bt)

    def d_drain(u):
        if u.pend_d is None:
            return
        ub, bt = u.pend_d
        u.pend_d = None
        ie = u.Ie
        if ub == 0:
            nc.vector.tensor_copy(u.S[:, 0:ie], bt[:, 0:ie])
        else:
            nc.vector.tensor_add(u.S[:, 0:ie], u.S[:, 0:ie], bt[:, 0:ie])
        if ub == 1:
            nc.vector.tensor_copy(u.Tt[:, 0:ie], bt[:, 0:ie])
        elif ub > 1:
            nc.vector.scalar_tensor_tensor(u.Tt[:, 0:ie], in0=bt[:, 0:ie],
                                           scalar=float(ub), in1=u.Tt[:, 0:ie],
                                           op0=ALU.mult, op1=ALU.add)

    def d_finalize(u):
        ie = u.Ie
        th = ps_mm.tile([128, 1024], F32, tag="mm", name=f"th_{u.k}")
        for off, w in _chunks(ie):
            nc.tensor.matmul(th[0:2, off:off + w], lhsT=thin_w[:, 0:2],
                             rhs=u.S[:, off:off + w], start=True, stop=True)
            nc.tensor.matmul(th[32:33, off:off + w], lhsT=thin_w[:, 2:3],
                             rhs=u.Tt[:, off:off + w], start=True, stop=True)
        for q, (off, w) in enumerate(_chunks(ie)):
            nc.vector.tensor_copy(th_sb[0:2, u.k, q, 0:w], th[0:2, off:off + w])
            nc.vector.tensor_copy(th_sb[32:33, u.k, q, 0:w],
                                  th[32:33, off:off + w])

    def segment(d_units, a_units):
        for u in a_units:
            u.nn = ps_nn.tile([128, 1024], F32, tag="nn", name=f"nn_{u.k}")
            u.dacc = daccp.tile([128, 1024], BF16, tag="dacc", name=f"dacc_{u.k}")
        for u in d_units:
            u.S = stp.tile([128, 1024], BF16, tag="S", name=f"S_{u.k}")
            u.Tt = stp.tile([128, 1024], BF16, tag="T", name=f"T_{u.k}")
        n_iter = max([u.T for u in a_units] + [u.I for u in d_units] + [0])
        for i in range(n_iter):
            for u in a_units:
                if i < u.T:
                    a_iter(u, i)
            for u in d_units:
                if i < u.I:
                    d_iter(u, i)
        for u in a_units:
            a_drain(u)
        for u in d_units:
            d_drain(u)
            d_finalize(u)

    units = []
    for s in range(SLOTS):
        cb, sb = plans[s]
        units.append(U(s, 0, sb, cb))
        units.append(U(s, 1, cb, sb))

    prev = []
    for u in units:
        segment(prev, [u])
        c_phase(u)
        prev = [u]
    segment(prev, [])

    # ---- final ----
    den8 = fin.tile([2 * SLOTS, 2, 512], F32, tag="den8")
    hi8 = fin.tile([2 * SLOTS, 2, 512], F32, tag="hi8")
    lo8 = fin.tile([2 * SLOTS, 2, 512], F32, tag="lo8")
    nc.sync.dma_start(out=den8, in_=th_sb[0:1, :, :, :])
    nc.sync.dma_start(out=hi8, in_=th_sb[1:2, :, :, :])
    nc.sync.dma_start(out=lo8, in_=th_sb[32:33, :, :, :])
    num8 = fin.tile([2 * SLOTS, 2, 512], F32, tag="num8")
    nc.vector.tensor_add(num8, hi8, lo8)
    rden = fin.tile([2 * SLOTS, 2, 512], F32, tag="rden8")
    scr = fin.tile([2 * SLOTS, 2, 512], F32, tag="scr")
    nc.vector.reciprocal_approx_accurate(out=rden, in_=den8, scratch=scr)
    idx = fin.tile([2 * SLOTS, 2, 512], F32, tag="idx")
    nc.vector.tensor_mul(idx, num8, rden)
    ierr = fin.tile([2 * SLOTS, 2, 512], F32, tag="ierr")
    nc.vector.tensor_sub(ierr, idx, iota_t)
    tmp = fin.tile([2 * SLOTS, 2, 512], F32, tag="tmp")
    nc.vector.tensor_mul(tmp, ierr, masks_t)
    sq = fin.tile([2 * SLOTS, 2, 512], F32, tag="sq")
    sums = fin.tile([2 * SLOTS, 1], F32, tag="sums")
    nc.vector.scalar_tensor_tensor(sq, in0=tmp, scalar=1.0, in1=ierr,
                                   op0=ALU.bypass, op1=ALU.mult, accum_out=sums)
    loss = fin.tile([2 * SLOTS, 1], F32, tag="loss")
    nc.vector.tensor_mul(loss, sums, rlens_t)
    nc.sync.dma_start(out=io["loss8"], in_=loss)


def _build_program(plans):
    key = tuple(plans)
    if key in _PROGRAM_CACHE:
        return _PROGRAM_CACHE[key]
    nc = bacc.Bacc("TRN2", target_bir_lowering=False, debug=False,
                   num_devices=NCORES)
    io = {
        "cembT": nc.dram_tensor("cembT", [SLOTS, D, M], BF16, kind="ExternalInput").ap(),
        "sembT": nc.dram_tensor("sembT", [SLOTS, D, N], BF16, kind="ExternalInput").ap(),
        "cembN": nc.dram_tensor("cembN", [SLOTS, 128, NB, D], BF16, kind="ExternalInput").ap(),
        "sembN": nc.dram_tensor("sembN", [SLOTS, 128, NB, D], BF16, kind="ExternalInput").ap(),
        "bias_c": nc.dram_tensor("bias_c", [SLOTS, 128, NB], F32, kind="ExternalInput").ap(),
        "bias_s": nc.dram_tensor("bias_s", [SLOTS, 128, NB], F32, kind="ExternalInput").ap(),
        "thin_w": nc.dram_tensor("thin_w", [128, 3], BF16, kind="ExternalInput").ap(),
        "iota8": nc.dram_tensor("iota8", [2 * SLOTS, M], F32, kind="ExternalInput").ap(),
        "masks8": nc.dram_tensor("masks8", [2 * SLOTS, M], F32, kind="ExternalInput").ap(),
        "rlens": nc.dram_tensor("rlens", [2 * SLOTS, 1], F32, kind="ExternalInput").ap(),
        "loss8": nc.dram_tensor("loss8", [2 * SLOTS, 1], F32, kind="ExternalOutput").ap(),
    }
    from contextlib import ExitStack
    with tile.TileContext(nc) as tc:
        with ExitStack() as ctx:
            _emit(nc, tc, ctx, io, plans)
    nc.compile()
    _PROGRAM_CACHE[key] = nc
    return nc


def _pick_order(cb_all, sb_all):
    """Pick the batch ordering minimizing total per-slot-max cost."""
    cost = lambda g: (2 * cb_all[g].max() * sb_all[g].max()
                      + cb_all[g].max() ** 2 + sb_all[g].max() ** 2)
    best, besto = None, None
    for key in [-(cb_all + sb_all) * 1000 - cb_all,
                -(sb_all * 16 + cb_all),
                -(cb_all * 16 + sb_all),
                -np.maximum(cb_all, sb_all) * 16 - (cb_all + sb_all)]:
        o = np.argsort(key, kind="stable")
        c = sum(cost(o[8 * s:8 * s + 8]) for s in range(SLOTS))
        if best is None or c < best:
            best, besto = c, o
    return besto


def _host_prep(clip_emb, clip_mask, clip_lens, sent_emb, sent_mask, sent_lens):
    """Batch->(core,slot) assignment, per-slot plans, per-core inputs."""
    mdt = ml_dtypes.bfloat16

    cb_all = np.ceil(clip_lens / 128).astype(int)
    sb_all = np.ceil(sent_lens / 128).astype(int)
    order = _pick_order(cb_all, sb_all)
    plans = []
    assign = {}  # (core, slot) -> batch
    for s in range(SLOTS):
        grp = order[8 * s:8 * s + 8]
        plans.append((int(cb_all[grp].max()), int(sb_all[grp].max())))
        for core, b in enumerate(grp):
            assign[(core, s)] = int(b)

    sq_c = np.einsum("bmd,bmd->bm", clip_emb, clip_emb)
    sq_s = np.einsum("bnd,bnd->bn", sent_emb, sent_emb)
    bias_c = (-sq_c / D + PEN * (1.0 - clip_mask)).astype(np.float32)
    bias_s = (-sq_s / D + PEN * (1.0 - sent_mask)).astype(np.float32)

    # thin weights: [ones | p | 128] (den2 | sum p*S | 128*sum T)
    thin_w = np.zeros((128, 3), np.float32)
    thin_w[:, 0] = 1.0
    thin_w[:, 1] = np.arange(128, dtype=np.float32)
    thin_w[:, 2] = 128.0
    iota8 = np.broadcast_to(np.arange(M, dtype=np.float32), (2 * SLOTS, M)).copy()

    in_maps = []
    for core in range(NCORES):
        bs = [assign[(core, s)] for s in range(SLOTS)]
        ce = clip_emb[bs]
        se = sent_emb[bs]
        masks8 = np.empty((2 * SLOTS, M), np.float32)
        rlens = np.empty((2 * SLOTS, 1), np.float32)
        for s, b in enumerate(bs):
            masks8[2 * s + 0] = clip_mask[b]
            masks8[2 * s + 1] = sent_mask[b]
            rlens[2 * s + 0] = 1.0 / clip_lens[b]
            rlens[2 * s + 1] = 1.0 / sent_lens[b]
        in_maps.append({
            "cembT": np.ascontiguousarray(ce.transpose(0, 2, 1)).astype(mdt),
            "sembT": np.ascontiguousarray(se.transpose(0, 2, 1)).astype(mdt),
            "cembN": np.ascontiguousarray(
                ce.reshape(SLOTS, NB, 128, D).transpose(0, 2, 1, 3)).astype(mdt),
            "sembN": np.ascontiguousarray(
                se.reshape(SLOTS, NB, 128, D).transpose(0, 2, 1, 3)).astype(mdt),
            "bias_c": np.ascontiguousarray(
                bias_c[bs].reshape(SLOTS, NB, 128).transpose(0, 2, 1)),
            "bias_s": np.ascontiguousarray(
                bias_s[bs].reshape(SLOTS, NB, 128).transpose(0, 2, 1)),
            "thin_w": thin_w.astype(mdt),
            "iota8": iota8,
            "masks8": masks8,
            "rlens": rlens,
        })
    return in_maps, assign, plans


def kernel(clip_emb, clip_mask, clip_lens, sent_emb, sent_mask, sent_lens):
    global LAST_RESULT
    clip_emb = np.asarray(clip_emb, np.float32)
    sent_emb = np.asarray(sent_emb, np.float32)
    clip_mask = np.asarray(clip_mask, np.float32)
    sent_mask = np.asarray(sent_mask, np.float32)
    clip_lens = np.asarray(clip_lens, np.float32)
    sent_lens = np.asarray(sent_lens, np.float32)

    in_maps, _, plans = _host_prep(clip_emb, clip_mask, clip_lens,
                                   sent_emb, sent_mask, sent_lens)
    nc = _build_program(plans)
    res = run_bass_kernel_spmd(nc, in_maps, list(range(NCORES)))
    LAST_RESULT = res

    rows = np.stack([res.results[c]["loss8"].reshape(2 * SLOTS) for c in range(NCORES)])
    clip_loss = rows[:, 0::2].mean()
    sent_loss = rows[:, 1::2].mean()
    return (np.float32(clip_loss), np.float32(sent_loss))


# revision 8
# speedup vs baseline: 1.2128x; 1.1925x over previous
"""CycleConsistencyLoss on 8 Trainium2 NeuronCores (Bass/Tile, SPMD data-parallel).

Math (per batch, clip [M,D], sent [N,D], prefix masks):
  soft_nn(src,tgt): w = softmax_j(-dist(src_i,tgt_j) masked); nn = w @ tgt
  dist = (|s|^2+|t|^2-2 s.t)/D; softmax shift-invariance =>
  w[i,j] prop exp((2 s_i.t_j - |t_j|^2)/D) * mask_j  (row terms cancel)
  index_nn = sum_u u*beta / sum_u beta over tgt2 = src embeddings
  loss_c = mean_b sum_i (index_nn[i]-i)^2 * mask_i / len_b

Device pipeline per (slot, cycle) unit, engines balanced:
  A: dots[t,i] = X^T Y (PE, full-width psum [128,<=1024]); Et = exp (one wide
     ACT op per t-block, bias folds -|t|^2/D - 1e4*(1-mask));
     nn[d,i] += Xn^T Et (PE); den += ones^T Et (PE thin matmul, psum row)
  C: rden = recip(den row) (DVE); bc = bcast (GPSIMD);
     nns = nn * bc (DVE, psum->sbuf bf16)
  D (ub descending): dots2[u,i] = Y^T nns (PE); Bt = exp (ACT);
     S += Bt; T += S (suffix-sum => sum ub*Bt, plain bf16 adds on DVE)
  fin: den2/num via 2 thin matmuls vs [1|p|128] weights (PE), rows staged
     by DMA, batched loss math, DMA out; host averages.

Per-slot length specialization: batches packed into slots minimizing
sum of per-slot (maxcb,maxsb) cost; block counts from slot maxima (SPMD-safe).
"""
import sys

sys.path.insert(0, "/opt/trn_rl_repo")

import numpy as np
import ml_dtypes

import concourse.bass as bass
import concourse.tile as tile
from concourse import bacc, bass_isa, mybir
from concourse.bass_utils import run_bass_kernel_spmd

F32 = mybir.dt.float32
BF16 = mybir.dt.bfloat16
EXP = mybir.ActivationFunctionType.Exp
ALU = mybir.AluOpType

B, M, N, D = 32, 1024, 1024, 128
NB = M // 128
NCORES = 8
SLOTS = B // NCORES  # 4
PEN = -1.0e4  # exp(PEN + x) == 0.0 exactly in fp32
SCALE = 2.0 / D

_PROGRAM_CACHE = {}
LAST_RESULT = None


def _chunks(ie):
    """Split extent ie into <=512-wide chunks (psum-bank sized)."""
    if ie <= 512:
        return [(0, ie)]
    return [(0, 512), (512, ie - 512)]


class U:
    """Per-(slot,cycle) unit state."""

    def __init__(self, s, c, n_tb, n_ub):
        self.s, self.c, self.k = s, c, 2 * s + c
        self.T, self.I = n_tb, n_ub
        self.Ie = 128 * n_ub
        self.pend_a = None
        self.pend_d = None


def _emit(nc, tc, ctx, io, plans):
    ts = bass.ts

    const = ctx.enter_context(tc.tile_pool(name="const", bufs=1))
    emb = ctx.enter_context(tc.tile_pool(name="emb", bufs=2))
    etp = ctx.enter_context(tc.tile_pool(name="etp", bufs=6))
    stp = ctx.enter_context(tc.tile_pool(name="stp", bufs=3))
    bcp = ctx.enter_context(tc.tile_pool(name="bcp", bufs=2))
    nnsp = ctx.enter_context(tc.tile_pool(name="nnsp", bufs=3))
    rrp = ctx.enter_context(tc.tile_pool(name="rrp", bufs=2))
    fin = ctx.enter_context(tc.tile_pool(name="fin", bufs=1))

    ps_mm = ctx.enter_context(tc.tile_pool(name="ps_mm", bufs=2, space="PSUM"))
    ps_nn = ctx.enter_context(tc.tile_pool(name="ps_nn", bufs=1, space="PSUM"))
    ps_dn = ctx.enter_context(tc.tile_pool(name="ps_dn", bufs=1, space="PSUM"))
    dn_tile = ps_dn.tile([97, 1024], F32, tag="dn")

    thin_w = const.tile([128, 3], BF16, tag="thin_w")
    nc.sync.dma_start(out=thin_w, in_=io["thin_w"])
    iota_t = const.tile([2 * SLOTS, 2, 512], F32, tag="iota")
    nc.sync.dma_start(out=iota_t, in_=io["iota8"].rearrange("r (q x) -> r q x", q=2))
    masks_t = const.tile([2 * SLOTS, 2, 512], F32, tag="masks")
    nc.sync.dma_start(out=masks_t, in_=io["masks8"].rearrange("r (q x) -> r q x", q=2))
    rlens_t = const.tile([2 * SLOTS, 1], F32, tag="rlens")
    nc.sync.dma_start(out=rlens_t, in_=io["rlens"])

    # staging: den2/numA at partitions 0/1, numB at partition 32 (engine ops
    # need start partition in {0,32,64,96}); memset 1.0 covers unwritten cols
    th_sb = fin.tile([33, 2 * SLOTS, 2, 512], F32, tag="th_sb")
    nc.vector.memset(th_sb, 1.0)

    slot_tiles = {}

    def get_slot(s):
        if s in slot_tiles:
            return slot_tiles[s]
        t = {}
        t["ct"] = emb.tile([128, M], BF16, tag="ct", name=f"ct{s}")
        nc.sync.dma_start(out=t["ct"], in_=io["cembT"][s])
        t["st"] = emb.tile([128, N], BF16, tag="st", name=f"st{s}")
        nc.sync.dma_start(out=t["st"], in_=io["sembT"][s])
        t["cn"] = emb.tile([128, NB, D], BF16, tag="cn", name=f"cn{s}")
        nc.sync.dma_start(out=t["cn"], in_=io["cembN"][s])
        t["sn"] = emb.tile([128, NB, D], BF16, tag="sn", name=f"sn{s}")
        nc.sync.dma_start(out=t["sn"], in_=io["sembN"][s])
        t["bias_c"] = emb.tile([128, NB], F32, tag="bias_c", name=f"bc{s}")
        nc.sync.dma_start(out=t["bias_c"], in_=io["bias_c"][s])
        t["bias_s"] = emb.tile([128, NB], F32, tag="bias_s", name=f"bs{s}")
        nc.sync.dma_start(out=t["bias_s"], in_=io["bias_s"][s])
        slot_tiles[s] = t
        return t

    def a_iter(u, tb):
        """dots(tb) + exp; nn/dacc of the PREVIOUS tb (keeps PE off ACT's back)."""
        t = get_slot(u.s)
        X = t["st"] if u.c == 0 else t["ct"]
        Y = t["ct"] if u.c == 0 else t["st"]
        b_tgt = t["bias_s"] if u.c == 0 else t["bias_c"]
        ie = u.Ie
        mm = ps_mm.tile([128, 1024], F32, tag="mm", name=f"mmA_{u.k}_{tb}")
        for off, w in _chunks(ie):
            nc.tensor.matmul(mm[:, off:off + w], lhsT=X[:, ts(tb, 128)],
                             rhs=Y[:, off:off + w], start=True, stop=True)
        et = etp.tile([128, 1024], BF16, tag="et", name=f"etA_{u.k}_{tb}")
        nc.scalar.activation(et[:, 0:ie], mm[:, 0:ie], EXP,
                             bias=b_tgt[:, tb:tb + 1], scale=SCALE)
        a_drain(u)
        u.pend_a = (tb, et)

    def a_drain(u):
        if u.pend_a is None:
            return
        tb, et = u.pend_a
        u.pend_a = None
        t = get_slot(u.s)
        Xn = t["sn"] if u.c == 0 else t["cn"]
        ie = u.Ie
        p0 = 32 * (u.k % 3)
        for off, w in _chunks(ie):
            nc.tensor.matmul(u.nn[:, off:off + w], lhsT=Xn[:, tb, :],
                             rhs=et[:, off:off + w],
                             start=(tb == 0), stop=(tb == u.T - 1))
            nc.tensor.matmul(dn_tile[p0:p0 + 1, off:off + w],
                             lhsT=thin_w[:, 0:1], rhs=et[:, off:off + w],
                             start=(tb == 0), stop=(tb == u.T - 1))

    def c_phase(u):
        ie = u.Ie
        p0 = 32 * (u.k % 3)
        rr = rrp.tile([1, 1024], F32, tag="rr", name=f"rr_{u.k}")
        nc.vector.reciprocal_approx_fast(out=rr[:, 0:ie],
                                         in_=dn_tile[p0:p0 + 1, 0:ie])
        bc = bcp.tile([128, 1024], F32, tag="bc", name=f"bc_{u.k}")
        nc.gpsimd.partition_broadcast(bc[:, 0:ie], rr[:, 0:ie])
        nns = nnsp.tile([128, 1024], BF16, tag="nns", name=f"nns_{u.k}")
        nc.vector.scalar_tensor_tensor(nns[:, 0:ie], in0=u.nn[:, 0:ie],
                                       scalar=1.0, in1=bc[:, 0:ie],
                                       op0=ALU.bypass, op1=ALU.mult)
        u.nns = nns

    def d_iter(u, ub):
        t = get_slot(u.s)
        Y = t["ct"] if u.c == 0 else t["st"]
        b_src = t["bias_c"] if u.c == 0 else t["bias_s"]
        ie = u.Ie
        mm2 = ps_mm.tile([128, 1024], F32, tag="mm", name=f"mmD_{u.k}_{ub}")
        for off, w in _chunks(ie):
            nc.tensor.matmul(mm2[:, off:off + w], lhsT=Y[:, ts(ub, 128)],
                             rhs=u.nns[:, off:off + w], start=True, stop=True)
        bt = etp.tile([128, 1024], BF16, tag="et", name=f"btD_{u.k}_{ub}")
        nc.scalar.activation(bt[:, 0:ie], mm2[:, 0:ie], EXP,
                             bias=b_src[:, ub:ub + 1], scale=SCALE)
        d_drain(u)
        u.pend_d = (ub, bt)

    def d_drain(u):
        # ub runs DESCENDING (I-1..0).  S = suffix sum of Bt; T accumulates
        # each suffix P_j for j>=1, so T = sum_j j*Bt_j without scalar muls.
        if u.pend_d is None:
            return
        ub, bt = u.pend_d
        u.pend_d = None
        ie = u.Ie
        if ub == u.I - 1:
            nc.vector.tensor_copy(u.S[:, 0:ie], bt[:, 0:ie])
            if ub >= 1:
                nc.vector.tensor_copy(u.Tt[:, 0:ie], u.S[:, 0:ie])
        else:
            nc.vector.tensor_add(u.S[:, 0:ie], u.S[:, 0:ie], bt[:, 0:ie])
            if ub >= 1:
                nc.vector.tensor_add(u.Tt[:, 0:ie], u.Tt[:, 0:ie], u.S[:, 0:ie])

    def d_finalize(u):
        ie = u.Ie
        th = ps_mm.tile([128, 1024], F32, tag="mm", name=f"th_{u.k}")
        for off, w in _chunks(ie):
            nc.tensor.matmul(th[0:2, off:off + w], lhsT=thin_w[:, 0:2],
                             rhs=u.S[:, off:off + w], start=True, stop=True)
            nc.tensor.matmul(th[32:33, off:off + w], lhsT=thin_w[:, 2:3],
                             rhs=u.Tt[:, off:off + w], start=True, stop=True)
        for q, (off, w) in enumerate(_chunks(ie)):
            nc.vector.tensor_copy(th_sb[0:2, u.k, q, 0:w], th[0:2, off:off + w])
            nc.vector.tensor_copy(th_sb[32:33, u.k, q, 0:w],
                                  th[32:33, off:off + w])

    def segment(d_units, a_units):
        for u in a_units:
            u.nn = ps_nn.tile([128, 1024], F32, tag="nn", name=f"nn_{u.k}")
        for u in d_units:
            u.S = stp.tile([128, 1024], BF16, tag="S", name=f"S_{u.k}")
            u.Tt = stp.tile([128, 1024], BF16, tag="T", name=f"T_{u.k}")
        n_iter = max([u.T for u in a_units] + [u.I for u in d_units] + [0])
        for i in range(n_iter):
            for u in a_units:
                if i < u.T:
                    a_iter(u, i)
            for u in d_units:
                if i < u.I:
                    d_iter(u, u.I - 1 - i)
        for u in a_units:
            a_drain(u)
        for u in d_units:
            d_drain(u)
            d_finalize(u)

    units = []
    for s in range(SLOTS):
        cb, sb = plans[s]
        units.append(U(s, 0, sb, cb))
        units.append(U(s, 1, cb, sb))

    prev = []
    for u in units:
        segment(prev, [u])
        c_phase(u)
        prev = [u]
    segment(prev, [])

    # ---- final ----
    den8 = fin.tile([2 * SLOTS, 2, 512], F32, tag="den8")
    hi8 = fin.tile([2 * SLOTS, 2, 512], F32, tag="hi8")
    lo8 = fin.tile([2 * SLOTS, 2, 512], F32, tag="lo8")
    nc.sync.dma_start(out=den8, in_=th_sb[0:1, :, :, :])
    nc.sync.dma_start(out=hi8, in_=th_sb[1:2, :, :, :])
    nc.sync.dma_start(out=lo8, in_=th_sb[32:33, :, :, :])
    num8 = fin.tile([2 * SLOTS, 2, 512], F32, tag="num8")
    nc.vector.tensor_add(num8, hi8, lo8)
    rden = fin.tile([2 * SLOTS, 2, 512], F32, tag="rden8")
    scr = fin.tile([2 * SLOTS, 2, 512], F32, tag="scr")
    nc.vector.reciprocal_approx_accurate(out=rden, in_=den8, scratch=scr)
    idx = fin.tile([2 * SLOTS, 2, 512], F32, tag="idx")
    nc.vector.tensor_mul(idx, num8, rden)
    ierr = fin.tile([2 * SLOTS, 2, 512], F32, tag="ierr")
    nc.vector.tensor_sub(ierr, idx, iota_t)
    tmp = fin.tile([2 * SLOTS, 2, 512], F32, tag="tmp")
    nc.vector.tensor_mul(tmp, ierr, masks_t)
    sq = fin.tile([2 * SLOTS, 2, 512], F32, tag="sq")
    sums = fin.tile([2 * SLOTS, 1], F32, tag="sums")
    nc.vector.scalar_tensor_tensor(sq, in0=tmp, scalar=1.0, in1=ierr,
                                   op0=ALU.bypass, op1=ALU.mult, accum_out=sums)
    loss = fin.tile([2 * SLOTS, 1], F32, tag="loss")
    nc.vector.tensor_mul(loss, sums, rlens_t)
    nc.sync.dma_start(out=io["loss8"], in_=loss)


def _build_program(plans):
    key = tuple(plans)
    if key in _PROGRAM_CACHE:
        return _PROGRAM_CACHE[key]
    nc = bacc.Bacc("TRN2", target_bir_lowering=False, debug=False,
                   num_devices=NCORES)
    io = {
        "cembT": nc.dram_tensor("cembT", [SLOTS, D, M], BF16, kind="ExternalInput").ap(),
        "sembT": nc.dram_tensor("sembT", [SLOTS, D, N], BF16, kind="ExternalInput").ap(),
        "cembN": nc.dram_tensor("cembN", [SLOTS, 128, NB, D], BF16, kind="ExternalInput").ap(),
        "sembN": nc.dram_tensor("sembN", [SLOTS, 128, NB, D], BF16, kind="ExternalInput").ap(),
        "bias_c": nc.dram_tensor("bias_c", [SLOTS, 128, NB], F32, kind="ExternalInput").ap(),
        "bias_s": nc.dram_tensor("bias_s", [SLOTS, 128, NB], F32, kind="ExternalInput").ap(),
        "thin_w": nc.dram_tensor("thin_w", [128, 3], BF16, kind="ExternalInput").ap(),
        "iota8": nc.dram_tensor("iota8", [2 * SLOTS, M], F32, kind="ExternalInput").ap(),
        "masks8": nc.dram_tensor("masks8", [2 * SLOTS, M], F32, kind="ExternalInput").ap(),
        "rlens": nc.dram_tensor("rlens", [2 * SLOTS, 1], F32, kind="ExternalInput").ap(),
        "loss8": nc.dram_tensor("loss8", [2 * SLOTS, 1], F32, kind="ExternalOutput").ap(),
    }
    from contextlib import ExitStack
    with tile.TileContext(nc) as tc:
        with ExitStack() as ctx:
            _emit(nc, tc, ctx, io, plans)
    nc.compile()
    _PROGRAM_CACHE[key] = nc
    return nc


def _pick_order(cb_all, sb_all):
    """Pick the batch ordering minimizing total per-slot-max cost."""
    cost = lambda g: (2 * cb_all[g].max() * sb_all[g].max()
                      + cb_all[g].max() ** 2 + sb_all[g].max() ** 2)
    best, besto = None, None
    for key in [-(cb_all + sb_all) * 1000 - cb_all,
                -(sb_all * 16 + cb_all),
                -(cb_all * 16 + sb_all),
                -np.maximum(cb_all, sb_all) * 16 - (cb_all + sb_all)]:
        o = np.argsort(key, kind="stable")
        c = sum(cost(o[8 * s:8 * s + 8]) for s in range(SLOTS))
        if best is None or c < best:
            best, besto = c, o
    return besto


def _host_prep(clip_emb, clip_mask, clip_lens, sent_emb, sent_mask, sent_lens):
    """Batch->(core,slot) assignment, per-slot plans, per-core inputs."""
    mdt = ml_dtypes.bfloat16

    cb_all = np.ceil(clip_lens / 128).astype(int)
    sb_all = np.ceil(sent_lens / 128).astype(int)
    order = _pick_order(cb_all, sb_all)
    plans = []
    assign = {}  # (core, slot) -> batch
    for s in range(SLOTS):
        grp = order[8 * s:8 * s + 8]
        plans.append((int(cb_all[grp].max()), int(sb_all[grp].max())))
        for core, b in enumerate(grp):
            assign[(core, s)] = int(b)

    sq_c = np.einsum("bmd,bmd->bm", clip_emb, clip_emb)
    sq_s = np.einsum("bnd,bnd->bn", sent_emb, sent_emb)
    bias_c = (-sq_c / D + PEN * (1.0 - clip_mask)).astype(np.float32)
    bias_s = (-sq_s / D + PEN * (1.0 - sent_mask)).astype(np.float32)

    # thin weights: [ones | p | 128] (den2 | sum p*S | 128*sum T)
    thin_w = np.zeros((128, 3), np.float32)
    thin_w[:, 0] = 1.0
    thin_w[:, 1] = np.arange(128, dtype=np.float32)
    thin_w[:, 2] = 128.0
    iota8 = np.broadcast_to(np.arange(M, dtype=np.float32), (2 * SLOTS, M)).copy()

    in_maps = []
    for core in range(NCORES):
        bs = [assign[(core, s)] for s in range(SLOTS)]
        ce = clip_emb[bs]
        se = sent_emb[bs]
        masks8 = np.empty((2 * SLOTS, M), np.float32)
        rlens = np.empty((2 * SLOTS, 1), np.float32)
        for s, b in enumerate(bs):
            masks8[2 * s + 0] = clip_mask[b]
            masks8[2 * s + 1] = sent_mask[b]
            rlens[2 * s + 0] = 1.0 / clip_lens[b]
            rlens[2 * s + 1] = 1.0 / sent_lens[b]
        in_maps.append({
            "cembT": np.ascontiguousarray(ce.transpose(0, 2, 1)).astype(mdt),
            "sembT": np.ascontiguousarray(se.transpose(0, 2, 1)).astype(mdt),
            "cembN": np.ascontiguousarray(
                ce.reshape(SLOTS, NB, 128, D).transpose(0, 2, 1, 3)).astype(mdt),
            "sembN": np.ascontiguousarray(
                se.reshape(SLOTS, NB, 128, D).transpose(0, 2, 1, 3)).astype(mdt),
            "bias_c": np.ascontiguousarray(
                bias_c[bs].reshape(SLOTS, NB, 128).transpose(0, 2, 1)),
            "bias_s": np.ascontiguousarray(
                bias_s[bs].reshape(SLOTS, NB, 128).transpose(0, 2, 1)),
            "thin_w": thin_w.astype(mdt),
            "iota8": iota8,
            "masks8": masks8,
            "rlens": rlens,
        })
    return in_maps, assign, plans


def kernel(clip_emb, clip_mask, clip_lens, sent_emb, sent_mask, sent_lens):
    global LAST_RESULT
    clip_emb = np.asarray(clip_emb, np.float32)
    sent_emb = np.asarray(sent_emb, np.float32)
    clip_mask = np.asarray(clip_mask, np.float32)
    sent_mask = np.asarray(sent_mask, np.float32)
    clip_lens = np.asarray(clip_lens, np.float32)
    sent_lens = np.asarray(sent_lens, np.float32)

    in_maps, _, plans = _host_prep(clip_emb, clip_mask, clip_lens,
                                   sent_emb, sent_mask, sent_lens)
    nc = _build_program(plans)
    res = run_bass_kernel_spmd(nc, in_maps, list(range(NCORES)))
    LAST_RESULT = res

    rows = np.stack([res.results[c]["loss8"].reshape(2 * SLOTS) for c in range(NCORES)])
    clip_loss = rows[:, 0::2].mean()
    sent_loss = rows[:, 1::2].mean()
    return (np.float32(clip_loss), np.float32(sent_loss))


# revision 10
# speedup vs baseline: 1.2327x; 1.0164x over previous
"""CycleConsistencyLoss on 8 Trainium2 NeuronCores (Bass/Tile, SPMD data-parallel).

Math (per batch, clip [M,D], sent [N,D], prefix masks):
  soft_nn(src,tgt): w = softmax_j(-dist(src_i,tgt_j) masked); nn = w @ tgt
  dist = (|s|^2+|t|^2-2 s.t)/D; softmax shift-invariance =>
  w[i,j] prop exp((2 s_i.t_j - |t_j|^2)/D) * mask_j  (row terms cancel)
  index_nn = sum_u u*beta / sum_u beta over tgt2 = src embeddings
  loss_c = mean_b sum_i (index_nn[i]-i)^2 * mask_i / len_b

Device pipeline per (slot, cycle) unit, engines balanced:
  A: dots[t,i] = X^T Y (PE, full-width psum [128,<=1024]); Et = exp (one wide
     ACT op per t-block, bias folds -|t|^2/D - 1e4*(1-mask));
     nn[d,i] += Xn^T Et (PE); den += ones^T Et (PE thin matmul, psum row)
  C: rden = recip(den row) (DVE); bc = bcast (GPSIMD);
     nns = nn * bc (DVE, psum->sbuf bf16)
  D (ub descending): dots2[u,i] = Y^T nns (PE); Bt = exp (ACT);
     S += Bt; T += S (suffix-sum => sum ub*Bt, plain bf16 adds on DVE)
  fin: den2/num via 2 thin matmuls vs [1|p|128] weights (PE), rows staged
     by DMA, batched loss math, DMA out; host averages.

Per-slot length specialization: batches packed into slots minimizing
sum of per-slot (maxcb,maxsb) cost; block counts from slot maxima (SPMD-safe).
"""
import sys

sys.path.insert(0, "/opt/trn_rl_repo")

import numpy as np
import ml_dtypes

import concourse.bass as bass
import concourse.tile as tile
from concourse import bacc, bass_isa, mybir
from concourse.bass_utils import run_bass_kernel_spmd

F32 = mybir.dt.float32
BF16 = mybir.dt.bfloat16
EXP = mybir.ActivationFunctionType.Exp
ALU = mybir.AluOpType

B, M, N, D = 32, 1024, 1024, 128
NB = M // 128
NCORES = 8
SLOTS = B // NCORES  # 4
PEN = -1.0e4  # exp(PEN + x) == 0.0 exactly in fp32
SCALE = 2.0 / D

_PROGRAM_CACHE = {}
LAST_RESULT = None


def _chunks(ie):
    """Split extent ie into <=512-wide chunks (psum-bank sized)."""
    if ie <= 512:
        return [(0, ie)]
    return [(0, 512), (512, ie - 512)]


class U:
    """Per-(slot,cycle) unit state."""

    def __init__(self, s, c, n_tb, n_ub):
        self.s, self.c, self.k = s, c, 2 * s + c
        self.T, self.I = n_tb, n_ub
        self.Ie = 128 * n_ub
        self.pend_a = None
        self.pend_d = None


def _emit(nc, tc, ctx, io, plans):
    ts = bass.ts

    const = ctx.enter_context(tc.tile_pool(name="const", bufs=1))
    emb = ctx.enter_context(tc.tile_pool(name="emb", bufs=3))
    etp = ctx.enter_context(tc.tile_pool(name="etp", bufs=8))
    stp = ctx.enter_context(tc.tile_pool(name="stp", bufs=3))
    bcp = ctx.enter_context(tc.tile_pool(name="bcp", bufs=2))
    nnsp = ctx.enter_context(tc.tile_pool(name="nnsp", bufs=3))
    rrp = ctx.enter_context(tc.tile_pool(name="rrp", bufs=2))
    fin = ctx.enter_context(tc.tile_pool(name="fin", bufs=1))

    ps_mm = ctx.enter_context(tc.tile_pool(name="ps_mm", bufs=2, space="PSUM"))
    ps_nn = ctx.enter_context(tc.tile_pool(name="ps_nn", bufs=1, space="PSUM"))
    ps_dn = ctx.enter_context(tc.tile_pool(name="ps_dn", bufs=1, space="PSUM"))
    dn_tile = ps_dn.tile([97, 1024], F32, tag="dn")

    thin_w = const.tile([128, 3], BF16, tag="thin_w")
    nc.sync.dma_start(out=thin_w, in_=io["thin_w"])
    iota_t = const.tile([2 * SLOTS, 2, 512], F32, tag="iota")
    nc.sync.dma_start(out=iota_t, in_=io["iota8"].rearrange("r (q x) -> r q x", q=2))
    masks_t = const.tile([2 * SLOTS, 2, 512], F32, tag="masks")
    nc.sync.dma_start(out=masks_t, in_=io["masks8"].rearrange("r (q x) -> r q x", q=2))
    rlens_t = const.tile([2 * SLOTS, 1], F32, tag="rlens")
    nc.sync.dma_start(out=rlens_t, in_=io["rlens"])

    # staging: den2/numA at partitions 0/1, numB at partition 32 (engine ops
    # need start partition in {0,32,64,96}); memset 1.0 covers unwritten cols
    th_sb = fin.tile([33, 2 * SLOTS, 2, 512], F32, tag="th_sb")
    nc.vector.memset(th_sb, 1.0)

    slot_tiles = {}

    def get_slot(s):
        if s in slot_tiles:
            return slot_tiles[s]
        t = {}
        t["ct"] = emb.tile([128, M], BF16, tag="ct", name=f"ct{s}")
        nc.sync.dma_start(out=t["ct"], in_=io["cembT"][s])
        t["st"] = emb.tile([128, N], BF16, tag="st", name=f"st{s}")
        nc.sync.dma_start(out=t["st"], in_=io["sembT"][s])
        t["cn"] = emb.tile([128, NB, D], BF16, tag="cn", name=f"cn{s}")
        nc.sync.dma_start(out=t["cn"], in_=io["cembN"][s])
        t["sn"] = emb.tile([128, NB, D], BF16, tag="sn", name=f"sn{s}")
        nc.sync.dma_start(out=t["sn"], in_=io["sembN"][s])
        t["bias_c"] = emb.tile([128, NB], F32, tag="bias_c", name=f"bc{s}")
        nc.sync.dma_start(out=t["bias_c"], in_=io["bias_c"][s])
        t["bias_s"] = emb.tile([128, NB], F32, tag="bias_s", name=f"bs{s}")
        nc.sync.dma_start(out=t["bias_s"], in_=io["bias_s"][s])
        slot_tiles[s] = t
        return t

    def a_iter(u, tb):
        """dots(tb) + exp; nn/dacc of the PREVIOUS tb (keeps PE off ACT's back)."""
        t = get_slot(u.s)
        X = t["st"] if u.c == 0 else t["ct"]
        Y = t["ct"] if u.c == 0 else t["st"]
        b_tgt = t["bias_s"] if u.c == 0 else t["bias_c"]
        ie = u.Ie
        mm = ps_mm.tile([128, 1024], F32, tag="mm", name=f"mmA_{u.k}_{tb}")
        for off, w in _chunks(ie):
            nc.tensor.matmul(mm[:, off:off + w], lhsT=X[:, ts(tb, 128)],
                             rhs=Y[:, off:off + w], start=True, stop=True)
        et = etp.tile([128, 1024], BF16, tag="et", name=f"etA_{u.k}_{tb}")
        nc.scalar.activation(et[:, 0:ie], mm[:, 0:ie], EXP,
                             bias=b_tgt[:, tb:tb + 1], scale=SCALE)
        a_drain(u)
        u.pend_a = (tb, et)

    def a_drain(u):
        if u.pend_a is None:
            return
        tb, et = u.pend_a
        u.pend_a = None
        t = get_slot(u.s)
        Xn = t["sn"] if u.c == 0 else t["cn"]
        ie = u.Ie
        p0 = 32 * (u.k % 3)
        for off, w in _chunks(ie):
            nc.tensor.matmul(u.nn[:, off:off + w], lhsT=Xn[:, tb, :],
                             rhs=et[:, off:off + w],
                             start=(tb == 0), stop=(tb == u.T - 1))
            nc.tensor.matmul(dn_tile[p0:p0 + 1, off:off + w],
                             lhsT=thin_w[:, 0:1], rhs=et[:, off:off + w],
                             start=(tb == 0), stop=(tb == u.T - 1))

    def c_phase(u):
        ie = u.Ie
        p0 = 32 * (u.k % 3)
        rr = rrp.tile([1, 1024], F32, tag="rr", name=f"rr_{u.k}")
        nc.vector.reciprocal_approx_fast(out=rr[:, 0:ie],
                                         in_=dn_tile[p0:p0 + 1, 0:ie])
        bc = bcp.tile([128, 1024], F32, tag="bc", name=f"bc_{u.k}")
        nc.gpsimd.partition_broadcast(bc[:, 0:ie], rr[:, 0:ie])
        nns = nnsp.tile([128, 1024], BF16, tag="nns", name=f"nns_{u.k}")
        nc.vector.scalar_tensor_tensor(nns[:, 0:ie], in0=u.nn[:, 0:ie],
                                       scalar=1.0, in1=bc[:, 0:ie],
                                       op0=ALU.bypass, op1=ALU.mult)
        u.nns = nns

    def d_iter(u, ub):
        t = get_slot(u.s)
        Y = t["ct"] if u.c == 0 else t["st"]
        b_src = t["bias_c"] if u.c == 0 else t["bias_s"]
        ie = u.Ie
        mm2 = ps_mm.tile([128, 1024], F32, tag="mm", name=f"mmD_{u.k}_{ub}")
        for off, w in _chunks(ie):
            nc.tensor.matmul(mm2[:, off:off + w], lhsT=Y[:, ts(ub, 128)],
                             rhs=u.nns[:, off:off + w], start=True, stop=True)
        bt = etp.tile([128, 1024], BF16, tag="et", name=f"btD_{u.k}_{ub}")
        nc.scalar.activation(bt[:, 0:ie], mm2[:, 0:ie], EXP,
                             bias=b_src[:, ub:ub + 1], scale=SCALE)
        d_drain(u)
        u.pend_d = (ub, bt)

    def d_drain(u):
        # ub runs DESCENDING (I-1..0).  S = suffix sum of Bt; T accumulates
        # each suffix P_j for j>=1, so T = sum_j j*Bt_j without scalar muls.
        if u.pend_d is None:
            return
        ub, bt = u.pend_d
        u.pend_d = None
        ie = u.Ie
        if ub == u.I - 1:
            nc.vector.tensor_copy(u.S[:, 0:ie], bt[:, 0:ie])
            if ub >= 1:
                nc.vector.tensor_copy(u.Tt[:, 0:ie], u.S[:, 0:ie])
        else:
            nc.vector.tensor_add(u.S[:, 0:ie], u.S[:, 0:ie], bt[:, 0:ie])
            if ub >= 1:
                nc.vector.tensor_add(u.Tt[:, 0:ie], u.Tt[:, 0:ie], u.S[:, 0:ie])

    def d_finalize(u):
        ie = u.Ie
        th = ps_mm.tile([128, 1024], F32, tag="mm", name=f"th_{u.k}")
        for off, w in _chunks(ie):
            nc.tensor.matmul(th[0:2, off:off + w], lhsT=thin_w[:, 0:2],
                             rhs=u.S[:, off:off + w], start=True, stop=True)
            nc.tensor.matmul(th[32:33, off:off + w], lhsT=thin_w[:, 2:3],
                             rhs=u.Tt[:, off:off + w], start=True, stop=True)
        for q, (off, w) in enumerate(_chunks(ie)):
            nc.vector.tensor_copy(th_sb[0:2, u.k, q, 0:w], th[0:2, off:off + w])
            nc.vector.tensor_copy(th_sb[32:33, u.k, q, 0:w],
                                  th[32:33, off:off + w])

    def segment(d_units, a_units, fin_units):
        for u in a_units:
            u.nn = ps_nn.tile([128, 1024], F32, tag="nn", name=f"nn_{u.k}")
        for u in d_units:
            u.S = stp.tile([128, 1024], BF16, tag="S", name=f"S_{u.k}")
            u.Tt = stp.tile([128, 1024], BF16, tag="T", name=f"T_{u.k}")
        n_iter = max([u.T for u in a_units] + [u.I for u in d_units] + [0])
        for i in range(n_iter):
            for u in a_units:
                if i < u.T:
                    a_iter(u, i)
                if i == u.T - 1:
                    # finish the A phase early: den is complete once nn/den
                    # of the last tb are emitted, so the recip/bcast/stt
                    # chain overlaps this segment's remaining D iterations
                    # and the nn psum buf frees before the next segment.
                    a_drain(u)
                    c_phase(u)
            if i == 1:
                # previous segment's D finalization: S/T are long complete,
                # and PE already has fresh dots work above it in the queue.
                for u in fin_units:
                    d_finalize(u)
            for u in d_units:
                if i < u.I:
                    d_iter(u, u.I - 1 - i)
        if n_iter <= 1:
            for u in fin_units:
                d_finalize(u)
        for u in d_units:
            d_drain(u)

    units = []
    for s in range(SLOTS):
        cb, sb = plans[s]
        units.append(U(s, 0, sb, cb))
        units.append(U(s, 1, cb, sb))

    prev, pend_fin = [], []
    for u in units:
        segment(prev, [u], pend_fin)
        pend_fin = prev
        prev = [u]
    segment(prev, [], pend_fin)
    for u in prev:
        d_finalize(u)

    # ---- final ----
    den8 = fin.tile([2 * SLOTS, 2, 512], F32, tag="den8")
    hi8 = fin.tile([2 * SLOTS, 2, 512], F32, tag="hi8")
    lo8 = fin.tile([2 * SLOTS, 2, 512], F32, tag="lo8")
    nc.sync.dma_start(out=den8, in_=th_sb[0:1, :, :, :])
    nc.sync.dma_start(out=hi8, in_=th_sb[1:2, :, :, :])
    nc.sync.dma_start(out=lo8, in_=th_sb[32:33, :, :, :])
    num8 = fin.tile([2 * SLOTS, 2, 512], F32, tag="num8")
    nc.vector.tensor_add(num8, hi8, lo8)
    rden = fin.tile([2 * SLOTS, 2, 512], F32, tag="rden8")
    scr = fin.tile([2 * SLOTS, 2, 512], F32, tag="scr")
    nc.vector.reciprocal_approx_accurate(out=rden, in_=den8, scratch=scr)
    idx = fin.tile([2 * SLOTS, 2, 512], F32, tag="idx")
    nc.vector.tensor_mul(idx, num8, rden)
    ierr = fin.tile([2 * SLOTS, 2, 512], F32, tag="ierr")
    nc.vector.tensor_sub(ierr, idx, iota_t)
    tmp = fin.tile([2 * SLOTS, 2, 512], F32, tag="tmp")
    nc.vector.tensor_mul(tmp, ierr, masks_t)
    sq = fin.tile([2 * SLOTS, 2, 512], F32, tag="sq")
    sums = fin.tile([2 * SLOTS, 1], F32, tag="sums")
    nc.vector.scalar_tensor_tensor(sq, in0=tmp, scalar=1.0, in1=ierr,
                                   op0=ALU.bypass, op1=ALU.mult, accum_out=sums)
    loss = fin.tile([2 * SLOTS, 1], F32, tag="loss")
    nc.vector.tensor_mul(loss, sums, rlens_t)
    nc.sync.dma_start(out=io["loss8"], in_=loss)


def _build_program(plans):
    key = tuple(plans)
    if key in _PROGRAM_CACHE:
        return _PROGRAM_CACHE[key]
    nc = bacc.Bacc("TRN2", target_bir_lowering=False, debug=False,
                   num_devices=NCORES)
    io = {
        "cembT": nc.dram_tensor("cembT", [SLOTS, D, M], BF16, kind="ExternalInput").ap(),
        "sembT": nc.dram_tensor("sembT", [SLOTS, D, N], BF16, kind="ExternalInput").ap(),
        "cembN": nc.dram_tensor("cembN", [SLOTS, 128, NB, D], BF16, kind="ExternalInput").ap(),
        "sembN": nc.dram_tensor("sembN", [SLOTS, 128, NB, D], BF16, kind="ExternalInput").ap(),
        "bias_c": nc.dram_tensor("bias_c", [SLOTS, 128, NB], F32, kind="ExternalInput").ap(),
        "bias_s": nc.dram_tensor("bias_s", [SLOTS, 128, NB], F32, kind="ExternalInput").ap(),
        "thin_w": nc.dram_tensor("thin_w", [128, 3], BF16, kind="ExternalInput").ap(),
        "iota8": nc.dram_tensor("iota8", [2 * SLOTS, M], F32, kind="ExternalInput").ap(),
        "masks8": nc.dram_tensor("masks8", [2 * SLOTS, M], F32, kind="ExternalInput").ap(),
        "rlens": nc.dram_tensor("rlens", [2 * SLOTS, 1], F32, kind="ExternalInput").ap(),
        "loss8": nc.dram_tensor("loss8", [2 * SLOTS, 1], F32, kind="ExternalOutput").ap(),
    }
    from contextlib import ExitStack
    with tile.TileContext(nc) as tc:
        with ExitStack() as ctx:
            _emit(nc, tc, ctx, io, plans)
    nc.compile()
    _PROGRAM_CACHE[key] = nc
    return nc


def _pick_order(cb_all, sb_all):
    """Pick the batch ordering minimizing total per-slot-max cost."""
    cost = lambda g: (2 * cb_all[g].max() * sb_all[g].max()
                      + cb_all[g].max() ** 2 + sb_all[g].max() ** 2)
    best, besto = None, None
    for key in [-(cb_all + sb_all) * 1000 - cb_all,
                -(sb_all * 16 + cb_all),
                -(cb_all * 16 + sb_all),
                -np.maximum(cb_all, sb_all) * 16 - (cb_all + sb_all)]:
        o = np.argsort(key, kind="stable")
        c = sum(cost(o[8 * s:8 * s + 8]) for s in range(SLOTS))
        if best is None or c < best:
            best, besto = c, o
    return besto


def _host_prep(clip_emb, clip_mask, clip_lens, sent_emb, sent_mask, sent_lens):
    """Batch->(core,slot) assignment, per-slot plans, per-core inputs."""
    mdt = ml_dtypes.bfloat16

    cb_all = np.ceil(clip_lens / 128).astype(int)
    sb_all = np.ceil(sent_lens / 128).astype(int)
    order = _pick_order(cb_all, sb_all)
    plans = []
    assign = {}  # (core, slot) -> batch
    for s in range(SLOTS):
        grp = order[8 * s:8 * s + 8]
        plans.append((int(cb_all[grp].max()), int(sb_all[grp].max())))
        for core, b in enumerate(grp):
            assign[(core, s)] = int(b)

    sq_c = np.einsum("bmd,bmd->bm", clip_emb, clip_emb)
    sq_s = np.einsum("bnd,bnd->bn", sent_emb, sent_emb)
    bias_c = (-sq_c / D + PEN * (1.0 - clip_mask)).astype(np.float32)
    bias_s = (-sq_s / D + PEN * (1.0 - sent_mask)).astype(np.float32)

    # thin weights: [ones | p | 128] (den2 | sum p*S | 128*sum T)
    thin_w = np.zeros((128, 3), np.float32)
    thin_w[:, 0] = 1.0
    thin_w[:, 1] = np.arange(128, dtype=np.float32)
    thin_w[:, 2] = 128.0
    iota8 = np.broadcast_to(np.arange(M, dtype=np.float32), (2 * SLOTS, M)).copy()

    in_maps = []
    for core in range(NCORES):
        bs = [assign[(core, s)] for s in range(SLOTS)]
        ce = clip_emb[bs]
        se = sent_emb[bs]
        masks8 = np.empty((2 * SLOTS, M), np.float32)
        rlens = np.empty((2 * SLOTS, 1), np.float32)
        for s, b in enumerate(bs):
            masks8[2 * s + 0] = clip_mask[b]
            masks8[2 * s + 1] = sent_mask[b]
            rlens[2 * s + 0] = 1.0 / clip_lens[b]
            rlens[2 * s + 1] = 1.0 / sent_lens[b]
        in_maps.append({
            "cembT": np.ascontiguousarray(ce.transpose(0, 2, 1)).astype(mdt),
            "sembT": np.ascontiguousarray(se.transpose(0, 2, 1)).astype(mdt),
            "cembN": np.ascontiguousarray(
                ce.reshape(SLOTS, NB, 128, D).transpose(0, 2, 1, 3)).astype(mdt),
            "sembN": np.ascontiguousarray(
                se.reshape(SLOTS, NB, 128, D).transpose(0, 2, 1, 3)).astype(mdt),
            "bias_c": np.ascontiguousarray(
                bias_c[bs].reshape(SLOTS, NB, 128).transpose(0, 2, 1)),
            "bias_s": np.ascontiguousarray(
                bias_s[bs].reshape(SLOTS, NB, 128).transpose(0, 2, 1)),
            "thin_w": thin_w.astype(mdt),
            "iota8": iota8,
            "masks8": masks8,
            "rlens": rlens,
        })
    return in_maps, assign, plans


def kernel(clip_emb, clip_mask, clip_lens, sent_emb, sent_mask, sent_lens):
    global LAST_RESULT
    clip_emb = np.asarray(clip_emb, np.float32)
    sent_emb = np.asarray(sent_emb, np.float32)
    clip_mask = np.asarray(clip_mask, np.float32)
    sent_mask = np.asarray(sent_mask, np.float32)
    clip_lens = np.asarray(clip_lens, np.float32)
    sent_lens = np.asarray(sent_lens, np.float32)

    in_maps, _, plans = _host_prep(clip_emb, clip_mask, clip_lens,
                                   sent_emb, sent_mask, sent_lens)
    nc = _build_program(plans)
    res = run_bass_kernel_spmd(nc, in_maps, list(range(NCORES)))
    LAST_RESULT = res

    rows = np.stack([res.results[c]["loss8"].reshape(2 * SLOTS) for c in range(NCORES)])
    clip_loss = rows[:, 0::2].mean()
    sent_loss = rows[:, 1::2].mean()
    return (np.float32(clip_loss), np.float32(sent_loss))
